# revision 3
# baseline (speedup 1.0000x reference)
"""Causal multi-head self-attention with RoPE on 8 Trainium2 NeuronCores.

Tensor-parallel over heads: each core handles 2 of 16 heads end-to-end
(QKV projection, RoPE, causal softmax attention, output projection with its
W_o row block). Host sums the 8 rank-128 partial outputs.

Device layouts (per core, per batch b):
  Q^T/K^T [128, 2048] f32r: rows = [h0:(even d | odd d), h1:(even d | odd d)]
  V       [128, 16, 2, 65] f32r: [t-part, t-block, head, (64 d | ones)]
  S^T     computed per (kb, q-chunk): rows k, cols q  (causal: q >= kb*128)
  out^T accum via [V|1] lhsT -> row 64 = softmax denominators
All matmuls in fp32r (11-bit mantissa inputs, fp32 accumulate).
"""
import numpy as np
from contextlib import ExitStack

import concourse.bass as bass
import concourse.tile as tile
from concourse import bacc, mybir
from concourse.bass_utils import run_bass_kernel_spmd

F32 = mybir.dt.float32
F32R = mybir.dt.float32r
AF = mybir.ActivationFunctionType

D, H, DK, T, B = 1024, 16, 64, 2048, 4
NCORES, HPC = 8, 2
NT = B * T
ROPE_THETA = 10000.0
_BUILT = {}


def _build_nc():
    nc = bacc.Bacc("TRN2", target_bir_lowering=False, debug=False,
                   num_devices=NCORES)
    xT = nc.dram_tensor("xT", [D, NT], F32R, kind="ExternalInput").ap()
    wq = nc.dram_tensor("wq", [D, 128], F32R, kind="ExternalInput").ap()
    wk = nc.dram_tensor("wk", [D, 128], F32R, kind="ExternalInput").ap()
    wv = nc.dram_tensor("wv", [D, 128], F32R, kind="ExternalInput").ap()
    wo = nc.dram_tensor("wo", [128, D], F32R, kind="ExternalInput").ap()
    cc = nc.dram_tensor("cc", [128, T], F32, kind="ExternalInput").ap()
    ss = nc.dram_tensor("ss", [128, T], F32, kind="ExternalInput").ap()
    tri = nc.dram_tensor("tri", [128, 128], F32R, kind="ExternalInput").ap()
    ident = nc.dram_tensor("ident", [128, 128], F32R, kind="ExternalInput").ap()
    ones = nc.dram_tensor("ones", [128, 32], F32R, kind="ExternalInput").ap()
    out = nc.dram_tensor("out", [NT, D], F32, kind="ExternalOutput").ap()

    x3 = xT.rearrange("(dt p) n -> p dt n", p=128)   # [128, 8, NT]
    wq3 = wq.rearrange("(dt p) m -> p dt m", p=128)  # [128, 8, 128]
    wk3 = wk.rearrange("(dt p) m -> p dt m", p=128)
    wv3 = wv.rearrange("(dt p) m -> p dt m", p=128)

    with tile.TileContext(nc) as tc, ExitStack() as ctx:
        consts = ctx.enter_context(tc.tile_pool(name="consts", bufs=1))
        wpool = ctx.enter_context(tc.tile_pool(name="wpool", bufs=1))
        xin = ctx.enter_context(tc.tile_pool(name="xin", bufs=2))
        qkv = ctx.enter_context(tc.tile_pool(name="qkv", bufs=2))
        rope = ctx.enter_context(tc.tile_pool(name="rope", bufs=2))
        ptp = ctx.enter_context(tc.tile_pool(name="ptp", bufs=3))
        nrm = ctx.enter_context(tc.tile_pool(name="nrm", bufs=2))
        osb = ctx.enter_context(tc.tile_pool(name="osb", bufs=3))
        ps_s = ctx.enter_context(tc.tile_pool(name="ps_s", bufs=2, space="PSUM"))
        ps_av = ctx.enter_context(tc.tile_pool(name="ps_av", bufs=1, space="PSUM"))
        ps_mix = ctx.enter_context(tc.tile_pool(name="ps_mix", bufs=2, space="PSUM"))

        w_q = wpool.tile([128, 8, 128], F32R)
        w_k = wpool.tile([128, 8, 128], F32R)
        w_v = wpool.tile([128, 8, 128], F32R)
        w_o = wpool.tile([128, D], F32R)
        c_cc = consts.tile([128, T], F32)
        c_ss = consts.tile([128, T], F32)
        c_tri = consts.tile([128, 128], F32R)
        c_id = consts.tile([128, 128], F32R)
        nc.sync.dma_start(w_q[:], wq3)
        nc.sync.dma_start(w_k[:], wk3)
        nc.sync.dma_start(w_v[:], wv3)
        nc.sync.dma_start(w_o[:], wo)
        nc.sync.dma_start(c_cc[:], cc)
        nc.sync.dma_start(c_ss[:], ss)
        nc.sync.dma_start(c_tri[:], tri)
        nc.sync.dma_start(c_id[:], ident)

        for b in range(B):
            # ---------------- Phase A: QKV^T projection + RoPE + V transpose
            qt = qkv.tile([128, T], F32R, tag="qt")
            kt = qkv.tile([128, T], F32R, tag="kt")
            vsb = qkv.tile([128, 16, 2, 65], F32R, tag="vsb")
            nc.sync.dma_start(vsb[:, :, :, 64:65],
                              ones.rearrange("p (g h o) -> p g h o", g=16, h=2))
            for tb in range(4):
                col0 = b * T + tb * 512
                lt = tb * 512
                xt = xin.tile([128, 8, 512], F32R, tag="xt")
                nc.sync.dma_start(xt[:], x3[:, :, col0:col0 + 512])
                for which, w_sb, dest in (("q", w_q, qt), ("k", w_k, kt),
                                          ("v", w_v, None)):
                    psA = ps_mix.tile([128, 512], F32, tag="mix")
                    for dt_i in range(8):
                        nc.tensor.matmul(psA[:], w_sb[:, dt_i, :], xt[:, dt_i, :],
                                         start=(dt_i == 0), stop=(dt_i == 7))
                    if which == "v":
                        vt = rope.tile([128, 512], F32R, tag="vt")
                        nc.any.tensor_copy(out=vt[:], in_=psA[:])
                        for s in range(4):
                            g = tb * 4 + s
                            ptr = ps_mix.tile([128, 128], F32R, tag="mix")
                            nc.tensor.transpose(ptr[:], vt[:, s * 128:(s + 1) * 128],
                                                c_id[:])
                            nc.any.tensor_copy(
                                out=vsb[:, g, :, 0:64],
                                in_=ptr[:].rearrange("p (h d) -> p h d", h=2))
                    else:
                        ta = rope.tile([128, 512], F32, tag="ropeA")
                        tb_ = rope.tile([128, 512], F32, tag="ropeB")
                        tsw = rope.tile([128, 512], F32, tag="ropeBsw")
                        nc.vector.tensor_mul(ta[:], psA[:], c_cc[:, lt:lt + 512])
                        nc.vector.tensor_mul(tb_[:], psA[:], c_ss[:, lt:lt + 512])
                        for hh in range(2):
                            r0 = hh * 64
                            nc.sync.dma_start(tsw[r0 + 32:r0 + 64, :],
                                              tb_[r0:r0 + 32, :])
                            nc.sync.dma_start(tsw[r0:r0 + 32, :],
                                              tb_[r0 + 32:r0 + 64, :])
                        nc.vector.tensor_add(dest[:, lt:lt + 512], ta[:], tsw[:])

            # ---------------- Phase B: causal attention per head
            attn = qkv.tile([128, T], F32R, tag="attnT")
            for h in range(HPC):
                r0 = h * 64
                for qh in range(2):  # pass over q halves [qh*1024, qh*1024+1024)
                    av = ps_av.tile([65, 1024], F32, tag="av")
                    for kb in range(8 * (qh + 1)):
                        k0 = kb * 128
                        q0 = max(qh * 1024, k0)
                        q1 = qh * 1024 + 1024
                        n = q1 - q0
                        sps = ps_s.tile([128, 1024], F32, tag="sps")
                        for half in range(2):
                            c0, c1 = q0 + half * 512, min(q0 + half * 512 + 512, q1)
                            if c0 >= c1:
                                continue
                            nc.tensor.matmul(
                                sps[:, c0 - q0:c1 - q0],
                                kt[r0:r0 + 64, k0:k0 + 128],
                                qt[r0:r0 + 64, c0:c1],
                                start=True, stop=True)
                        pt = ptp.tile([128, 1024], F32R, tag="pt")
                        nc.scalar.activation(pt[:, :n], sps[:, :n], AF.Exp)
                        if q0 == k0:  # diagonal block: causal triangle mask
                            nc.vector.tensor_mul(pt[:, 0:128], pt[:, 0:128],
                                                 c_tri[:])
                        for qc in (2 * qh, 2 * qh + 1):
                            c0 = max(qc * 512, q0)
                            c1 = qc * 512 + 512
                            if c0 >= c1:
                                continue
                            a0 = qc * 512 - qh * 1024
                            nc.tensor.matmul(
                                av[:, c0 - qh * 1024:c1 - qh * 1024],
                                vsb[:, kb, h, :],
                                pt[:, c0 - q0:c1 - q0],
                                start=(kb == 0), stop=(kb == 4 * qc + 3),
                                skip_group_check=True)
                            del a0
                            if kb == 4 * qc + 3:  # chunk complete -> normalize
                                o0 = qc * 512 - qh * 1024
                                rc = nrm.tile([128, 512], F32, tag="rc")
                                nc.vector.reciprocal(rc[64:65, :],
                                                     av[64:65, o0:o0 + 512])
                                rc0 = nrm.tile([1, 512], F32, tag="rc0")
                                nc.sync.dma_start(rc0[0:1, :], rc[64:65, :])
                                rb = nrm.tile([64, 512], F32, tag="rb")
                                nc.gpsimd.partition_broadcast(rb[0:64, :],
                                                              rc0[0:1, :])
                                if h == 0:
                                    nc.vector.tensor_mul(
                                        attn[0:64, qc * 512:qc * 512 + 512],
                                        av[0:64, o0:o0 + 512], rb[0:64, :])
                                else:
                                    a1 = nrm.tile([64, 512], F32R, tag="a1")
                                    nc.vector.tensor_mul(
                                        a1[0:64, :], av[0:64, o0:o0 + 512],
                                        rb[0:64, :])
                                    nc.sync.dma_start(
                                        attn[64:128, qc * 512:qc * 512 + 512],
                                        a1[0:64, :])

            # ---------------- Phase C: output projection (W_o row block)
            for tt in range(16):
                o_sb = osb.tile([128, D], F32, tag="osb")
                for h5 in range(2):
                    pso = ps_mix.tile([128, 512], F32, tag="mix")
                    nc.tensor.matmul(pso[:], attn[:, tt * 128:(tt + 1) * 128],
                                     w_o[:, h5 * 512:(h5 + 1) * 512],
                                     start=True, stop=True)
                    nc.any.tensor_copy(out=o_sb[:, h5 * 512:(h5 + 1) * 512],
                                       in_=pso[:])
                row = b * T + tt * 128
                nc.sync.dma_start(out[row:row + 128, :], o_sb[:])

    nc.compile()
    return nc


def _host_prep(x, W_qkv, W_o, token_positions):
    x = np.ascontiguousarray(np.asarray(x, np.float32))
    W_qkv = np.asarray(W_qkv, np.float32)
    W_o = np.asarray(W_o, np.float32)
    pos = np.asarray(token_positions, np.float64)
    xT = np.ascontiguousarray(x.reshape(NT, D).T)
    i = np.arange(32)
    inv = 1.0 / (ROPE_THETA ** (2 * i / DK))
    ang = pos[None, :] * inv[:, None]
    cos, sin = np.cos(ang), np.sin(ang)
    CC = np.tile(cos, (4, 1)).astype(np.float32)
    SS = np.concatenate([sin, -sin, sin, -sin], 0).astype(np.float32)
    tri = (np.arange(128)[:, None] <= np.arange(128)[None, :]).astype(np.float32)
    ident = np.eye(128, dtype=np.float32)
    in_maps = []
    for c in range(NCORES):
        qcols, vcols = [], []
        for h in range(HPC):
            hh = HPC * c + h
            for half in range(2):
                qcols.extend(hh * DK + 2 * ii + half for ii in range(32))
            vcols.extend(hh * DK + d for d in range(DK))
        qcols = np.array(qcols)
        vcols = np.array(vcols)
        in_maps.append({
            "xT": xT,
            "wq": np.ascontiguousarray(W_qkv[:, 0 * D + qcols]),
            "wk": np.ascontiguousarray(W_qkv[:, 1 * D + qcols] / 8.0),
            "wv": np.ascontiguousarray(W_qkv[:, 2 * D + vcols]),
            "wo": np.ascontiguousarray(W_o[vcols, :]),
            "cc": CC, "ss": SS, "tri": tri, "ident": ident,
            "ones": np.ones((128, 32), np.float32),
        })
    return in_maps


def kernel(x, W_qkv, W_o, token_positions, _trace=False):
    in_maps = _host_prep(x, W_qkv, W_o, token_positions)
    if "nc" not in _BUILT:
        _BUILT["nc"] = _build_nc()
    res = run_bass_kernel_spmd(_BUILT["nc"], in_maps,
                               core_ids=list(range(NCORES)), trace=_trace)
    _BUILT["last_result"] = res
    total = np.zeros((NT, D), np.float32)
    for r in res.results:
        total += r["out"]
    return total.reshape(B, T, D)
